# revision 31
# baseline (speedup 1.0000x reference)
"""LocallyConnectedXYZLayer Trainium2 kernel.

out[n,c,h,w] = sum_{dy,dx in 5x5} exp(-|xyz(n,:,h+dy-2,w+dx-2)-xyz(n,:,h,w)|^2/2)
               * (softmax*mask)(n,c,h+dy-2,w+dx-2)        (zero-padded)

Sharding: 8 cores = (batch n = core//2) x (W half = core%2).
Per-core layout: partitions = 2 w-chunks x 64 h rows; free dims = (dy, c, w).
The 5 dy window shifts are baked into host-prepared per-partition rows (one
DMA per tensor per step); dx shifts are free-dim slices.  Per 5x5 offset:
one bf16 tensor_tensor mul (gaussian broadcast over channels via a step-0 AP
dim) + one accumulate add on the vector engine; squared-distance chain runs
in fp32 on gpsimd, exp on the scalar engine; per-dx group sums in bf16 with
an fp32 master accumulator (hierarchical accumulation for precision).
"""

import sys
from contextlib import ExitStack

import numpy as np

sys.path.insert(0, "/opt/trn_rl_repo")

import ml_dtypes  # noqa: E402

import concourse.bass as bass  # noqa: E402
from concourse import mybir  # noqa: E402
from concourse.bass_utils import run_bass_kernel_spmd  # noqa: E402

BF16 = ml_dtypes.bfloat16

N, C, H, W = 4, 20, 64, 2048
KH = KW = 5
PAD = 2
HH = H + 2 * PAD  # 68 padded rows
WCORE = W // 2  # 1024 interior w per core
NSTEP = 4  # device steps
WS = WCORE // (2 * NSTEP)  # 128 interior w per (step, chunk)
WX = WS + 2 * PAD  # 132 w extent (halo 2 each side)

_CACHE = {}


def _build_nc():
    """Raw-Bass program (no Tile): this toolchain's walrus codegen allows at
    most one sync-wait command per instruction, so all cross-engine sync is
    standalone wait_ge instructions plus one then_inc on producer ops."""
    nc = bass.Bass("TRN2", target_bir_lowering=False, debug=False)
    bf = mybir.dt.bfloat16
    f32 = mybir.dt.float32
    sm_d = nc.dram_tensor("sm_in", [NSTEP, 128, KH, C, WX], bf,
                          kind="ExternalInput")
    xyz_d = nc.dram_tensor("xyz_in", [NSTEP, 128, KH, 3, WX], bf,
                           kind="ExternalInput")
    m_d = nc.dram_tensor("m_in", [NSTEP, 128, KH, WX], bf,
                         kind="ExternalInput")
    out_d = nc.dram_tensor("out_d", [NSTEP, 128, C, WS], f32,
                           kind="ExternalOutput")

    def sb(name, shape, dt):
        return nc.alloc_sbuf_tensor(name, list(shape), dt).ap()

    # double-buffered input tiles (per step parity)
    sm_t = [sb(f"sm{i}", [128, KH, C, WX], bf) for i in range(2)]
    xyz_t = [sb(f"xyz{i}", [128, KH, 3, WX], bf) for i in range(2)]
    m_t = [sb(f"m{i}", [128, KH, WX], bf) for i in range(2)]
    # d2 chain (gpsimd-internal reuse is in-order; d2 double-buffered for ACT)
    diff_t = sb("diff", [128, KH, 3, WS], f32)
    d2_t = [sb(f"d2_{i}", [128, KH, WS], f32) for i in range(2)]
    g5_t = [sb(f"g5_{i}", [128, KH, WS], bf) for i in range(2)]
    gm5_t = sb("gm5", [128, KH, WS], bf)
    t_t = sb("t", [128, C, WS], bf)
    group_t = sb("group", [128, C, WS], bf)
    master_t = [sb(f"master{i}", [128, C, WS], f32) for i in range(2)]

    ADD, MULT, SUB = (mybir.AluOpType.add, mybir.AluOpType.mult,
                      mybir.AluOpType.subtract)

    with ExitStack() as ctx:
        load_sem = ctx.enter_context(nc.semaphore("load_sem"))
        store_sem = ctx.enter_context(nc.semaphore("store_sem"))
        pool_sem = ctx.enter_context(nc.semaphore("pool_sem"))
        act_sem = ctx.enter_context(nc.semaphore("act_sem"))
        dve_sem = ctx.enter_context(nc.semaphore("dve_sem"))
        block = ctx.enter_context(nc.Block())

        @block.sync
        def _(sync):
            for s in range(NSTEP):
                b = s % 2
                if s >= 1:
                    # DMA completions across steps are unordered; gate this
                    # step's loads on the previous step's completions so the
                    # cumulative load_sem threshold implies the right data.
                    sync.wait_ge(load_sem, 48 * s)
                if s >= 2:
                    # input buffer reuse: step s-2 consumers must be done
                    sync.wait_ge(pool_sem, 20 * (s - 1))
                    sync.wait_ge(dve_sem, 55 * (s - 1))
                sync.dma_start(sm_t[b][:], sm_d[s]).then_inc(load_sem, 16)
                sync.dma_start(xyz_t[b][:], xyz_d[s]).then_inc(load_sem, 16)
                sync.dma_start(m_t[b][:], m_d[s]).then_inc(load_sem, 16)
                if s >= 1:
                    sync.wait_ge(dve_sem, 55 * s)
                    if s >= 2:
                        sync.wait_ge(store_sem, 16 * (s - 1))
                    sync.dma_start(out_d[s - 1],
                                   master_t[(s - 1) % 2][:]).then_inc(
                                       store_sem, 16)
            sync.wait_ge(dve_sem, 55 * NSTEP)
            sync.wait_ge(store_sem, 16 * (NSTEP - 1))
            sync.dma_start(out_d[NSTEP - 1],
                           master_t[(NSTEP - 1) % 2][:]).then_inc(
                               store_sem, 16)

        @block.gpsimd
        def _(gpsimd):
            # gpsimd executes with internal overlap: every dependent chain
            # step needs an explicit self-wait (4 pool ops per tau).
            for s in range(NSTEP):
                b = s % 2
                gpsimd.wait_ge(load_sem, 48 * (s + 1))
                xyz_c = xyz_t[b][:, 2, :, PAD:PAD + WS].unsqueeze(
                    1).broadcast_to([128, KH, 3, WS])
                for dx in range(KW):
                    tau = 5 * s + dx
                    if tau >= 1:
                        # diff buffer WAR: previous d2b must be done
                        gpsimd.wait_ge(pool_sem, 4 * tau)
                    if tau >= 2:
                        # d2 buffer reuse: exp of group tau-2 must be done
                        gpsimd.wait_ge(act_sem, tau - 1)
                    gpsimd.tensor_tensor(out=diff_t[:],
                                         in0=xyz_t[b][:, :, :, dx:dx + WS],
                                         in1=xyz_c,
                                         op=SUB).then_inc(pool_sem)
                    gpsimd.wait_ge(pool_sem, 4 * tau + 1)
                    gpsimd.tensor_tensor(out=diff_t[:], in0=diff_t[:],
                                         in1=diff_t[:],
                                         op=MULT).then_inc(pool_sem)
                    d2 = d2_t[tau % 2]
                    gpsimd.wait_ge(pool_sem, 4 * tau + 2)
                    gpsimd.tensor_tensor(out=d2[:], in0=diff_t[:, :, 0, :],
                                         in1=diff_t[:, :, 1, :],
                                         op=ADD).then_inc(pool_sem)
                    gpsimd.wait_ge(pool_sem, 4 * tau + 3)
                    gpsimd.tensor_tensor(out=d2[:], in0=d2[:],
                                         in1=diff_t[:, :, 2, :],
                                         op=ADD).then_inc(pool_sem)

        @block.scalar
        def _(scalar):
            for s in range(NSTEP):
                for dx in range(KW):
                    tau = 5 * s + dx
                    scalar.wait_ge(pool_sem, 4 * (tau + 1))
                    if tau >= 2:
                        # g5 buffer reuse: gm5 of group tau-2 must be done
                        scalar.wait_ge(dve_sem, 11 * (tau - 2) + 1)
                    scalar.activation(
                        out=g5_t[tau % 2][:], in_=d2_t[tau % 2][:],
                        func=mybir.ActivationFunctionType.Exp,
                        scale=-0.5).then_inc(act_sem)

        @block.vector
        def _(vector):
            # Every DVE op increments dve_sem; dependent ops are preceded by
            # a self-wait on the producer's count (in-order on HW, and keeps
            # CoreSim's race detector satisfied).
            nv = [0]

            def vop(bi):
                nv[0] += 1
                return bi

            def vwait():
                if nv[0] > 0:
                    vector.wait_ge(dve_sem, nv[0])

            for s in range(NSTEP):
                b = s % 2
                vector.wait_ge(load_sem, 48 * (s + 1))
                if s >= 2:
                    # master buffer reuse: store of step s-2 must be done
                    vector.wait_ge(store_sem, 16 * (s - 1))
                master = master_t[b]
                for dx in range(KW):
                    tau = 5 * s + dx
                    vector.wait_ge(act_sem, tau + 1)
                    vwait()
                    vop(vector.tensor_tensor(
                        out=gm5_t[:], in0=g5_t[tau % 2][:],
                        in1=m_t[b][:, :, dx:dx + WS],
                        op=MULT).then_inc(dve_sem))
                    for dy in range(KH):
                        sm_s = sm_t[b][:, dy, :, dx:dx + WS]
                        g_b = gm5_t[:, dy, :].unsqueeze(1).broadcast_to(
                            [128, C, WS])
                        if dy == 0:
                            vwait()
                            vop(vector.tensor_tensor(
                                out=group_t[:], in0=sm_s, in1=g_b,
                                op=MULT).then_inc(dve_sem))
                        else:
                            vwait()
                            vop(vector.tensor_tensor(
                                out=t_t[:], in0=sm_s, in1=g_b,
                                op=MULT).then_inc(dve_sem))
                            vwait()
                            vop(vector.tensor_tensor(
                                out=group_t[:], in0=group_t[:], in1=t_t[:],
                                op=ADD).then_inc(dve_sem))
                    vwait()
                    if dx == 0:
                        vop(vector.tensor_copy(
                            master[:], group_t[:]).then_inc(dve_sem))
                    else:
                        vop(vector.tensor_tensor(
                            out=master[:], in0=master[:], in1=group_t[:],
                            op=ADD).then_inc(dve_sem))

    return nc


def _prep_core(xyz, softmax, mask, core):
    """Build the per-core dy-baked slab arrays (host side, bf16).

    Row layout: partition p (0..127) = chunk (p//64) x h row (p%64); the
    dy dim holds the 5 shifted window rows h+dy (in padded coords)."""
    n, half = core // 2, core % 2
    w0 = WCORE * half
    wp_sz = WCORE + 2 * PAD
    lo, hi = w0 - PAD, w0 + WCORE + PAD
    glo, ghi = max(lo, 0), min(hi, W)

    smp = np.zeros((HH, C, wp_sz), BF16)
    smp[PAD:PAD + H, :, glo - lo:ghi - lo] = (
        softmax[n][:, :, glo:ghi].transpose(1, 0, 2).astype(BF16))
    xyzp = np.zeros((HH, 3, wp_sz), BF16)
    xyzp[PAD:PAD + H, :, glo - lo:ghi - lo] = (
        xyz[n][:, :, glo:ghi].transpose(1, 0, 2).astype(BF16))
    mp = np.zeros((HH, wp_sz), BF16)
    mp[PAD:PAD + H, glo - lo:ghi - lo] = mask[n][:, glo:ghi].astype(BF16)

    sm5 = np.empty((NSTEP, 128, KH, C, WX), BF16)
    xyz5 = np.empty((NSTEP, 128, KH, 3, WX), BF16)
    m5 = np.empty((NSTEP, 128, KH, WX), BF16)
    for s in range(NSTEP):
        for chunk in range(2):
            wb = WS * (s + NSTEP * chunk)
            pr = slice(64 * chunk, 64 * chunk + 64)
            for dy in range(KH):
                sm5[s, pr, dy] = smp[dy:dy + 64, :, wb:wb + WX]
                xyz5[s, pr, dy] = xyzp[dy:dy + 64, :, wb:wb + WX]
                m5[s, pr, dy] = mp[dy:dy + 64, wb:wb + WX]
    return {"sm_in": sm5, "xyz_in": xyz5, "m_in": m5}


def make_in_maps(xyz, softmax, mask):
    return [_prep_core(xyz, softmax, mask, k) for k in range(8)]


def assemble_out(results):
    out = np.empty((N, C, H, W), np.float32)
    for core in range(8):
        n, half = core // 2, core % 2
        w0 = WCORE * half
        o = np.asarray(results[core]["out_d"], dtype=np.float32)
        # [s, chunk*64+h, c, j] -> [c, h, (s + NSTEP*chunk)*WS + j]
        o = o.reshape(NSTEP, 2, H, C, WS)
        # -> [c, h, chunk, s, j]
        out[n, :, :, w0:w0 + WCORE] = o.transpose(3, 2, 1, 0, 4).reshape(
            C, H, WCORE)
    return out


def get_nc():
    if "nc" not in _CACHE:
        _CACHE["nc"] = _build_nc()
    return _CACHE["nc"]


def kernel(xyz, softmax, mask, trace=False, trace_kwargs=None):
    nc = get_nc()
    in_maps = make_in_maps(np.asarray(xyz), np.asarray(softmax),
                           np.asarray(mask))
    res = run_bass_kernel_spmd(nc, in_maps, list(range(8)), trace=trace,
                               **(trace_kwargs or {}))
    out = assemble_out(res.results)
    if trace:
        return out, res
    return out
